# revision 20
# baseline (speedup 1.0000x reference)
"""Trainium2 Bass kernel for nn_Attention_43516608643501.

Cross-attention: Q = out_d [T,B,H]; K = V = sum of fwd/bwd halves of out_e
-> [S,B,H]; scores = Q @ K^T per batch (contraction over H, no scaling);
softmax over the source dim S; context = P @ V -> output [T,B,H].

Sharding: data-parallel over batch (dim 1): 2 batches per core x 8 cores,
no cross-core communication.

v8 design:
- Batch 0 prep uses per-tile PE transposes (fp16 via identity) so compute
  starts ~10us in; batch 1 prep rides the DMA crossbar (fp16 DRAM scratch
  store + dma_start_transpose, all on the SP queue in FIFO order) while
  batch 0's main loop runs, costing the PE nothing. ACT-queue DMA is never
  used: its completion semaphores proved unreliable under tight
  consumption.
- Softmax scores live in two 1024-wide 2-bank PSUM tiles; exp runs as two
  activations whose accum_out gives the row sums.
- The t-tile pipeline is depth-2 and flattened across both batches. Issue
  order within an iteration: mm1(i) -> m/neg_m(i) -> exp(i) ->
  stage2(i-2) -> l/linv(i). This keeps the PE order [mm1(i), Ptr(i-2),
  mm2(i-2)] while placing the P PSUM->SBUF copies of tile i-2 BEFORE
  l/linv(i) in the DVE stream, so mm2 never waits behind the exp->l chain.

Numerics: fp16 matmuls (4.9e-4 rounding; scores carry no 1/sqrt(H)
scale, so softmax near-ties amplify score error by exp()). Per-row max
subtraction on the free dim keeps exp args <= 0.
"""

import numpy as np
from contextlib import ExitStack

S, T, B, H = 2048, 2048, 16, 512
NCORES = 8
BLOC = B // NCORES  # batches per core
P128 = 128
NS = S // P128  # 16 s-tiles
NT = T // P128  # 16 t-tiles
NH = H // P128  # 4 h-chunks of the contraction
SC = 512  # s-chunk width (scores tile columns)
NSC = S // SC  # 4 s-chunks per t-tile
GRP = 4  # s/t tiles per xbar group (512 rows)

_cached_nc = None


def _build():
    import concourse.bacc as bacc
    import concourse.tile as tile
    from concourse import mybir
    from concourse.masks import make_identity

    f32 = mybir.dt.float32
    f16 = mybir.dt.float16

    nc = bacc.Bacc(None, target_bir_lowering=False)
    d_oe = nc.dram_tensor("out_e", [S, BLOC, 2 * H], f32, kind="ExternalInput")
    d_od = nc.dram_tensor("out_d", [T, BLOC, H], f32, kind="ExternalInput")
    d_out = nc.dram_tensor("out", [T, BLOC, H], f32, kind="ExternalOutput")

    with ExitStack() as ctx:
        tc = ctx.enter_context(tile.TileContext(nc))
        singles = ctx.enter_context(tc.tile_pool(name="singles", bufs=1))
        loads = ctx.enter_context(tc.tile_pool(name="loads", bufs=6))
        persist = ctx.enter_context(tc.tile_pool(name="persist", bufs=2))
        odtw = ctx.enter_context(tc.tile_pool(name="odtw", bufs=1))
        ptile = ctx.enter_context(tc.tile_pool(name="ptile", bufs=3))
        pcpool = ctx.enter_context(tc.tile_pool(name="pcpool", bufs=2))
        outs = ctx.enter_context(tc.tile_pool(name="outs", bufs=3))
        small = ctx.enter_context(tc.tile_pool(name="small", bufs=5))
        dscratch = ctx.enter_context(
            tc.tile_pool(name="dscratch", bufs=1, space="DRAM")
        )
        # PSUM: 8 banks = scores halves (2x2) + transpose staging (2) +
        # context (2)
        ps_s_pool = ctx.enter_context(
            tc.tile_pool(name="ps_s_pool", bufs=1, space="PSUM")
        )
        ps_tr = ctx.enter_context(tc.tile_pool(name="ps_tr", bufs=2, space="PSUM"))
        ps_cp = ctx.enter_context(tc.tile_pool(name="ps_cp", bufs=2, space="PSUM"))

        id16 = singles.tile([P128, P128], f16)
        make_identity(nc, id16)

        # ---- batch 0: per-tile PE-transpose prep (v1-style, fast start) ----
        b0 = {
            "b": 0,
            "oe_nat": [None] * NS,
            "oeT_c": [
                persist.tile([P128, NH, SC], f16, tag=f"oeT{g}", name=f"oeT{g}")
                for g in range(NSC)
            ],
            "odT_t": [
                odtw.tile([P128, NH, P128], f16, tag=f"odTt{k}", name=f"odTt{k}")
                for k in range(NT)
            ],
        }
        b0["odw"] = lambda tt, hc: b0["odT_t"][tt][:, hc, :]

        def prep_oe0(k):
            raw = loads.tile([P128, 2 * H], f32, tag="raw", name="raw")
            nc.sync.dma_start(out=raw, in_=d_oe[k * P128:(k + 1) * P128, 0, :])
            nat = persist.tile([P128, H], f16, tag=f"oenat{k}", name=f"oenat{k}")
            nc.vector.tensor_add(nat, raw[:, 0:H], raw[:, H:2 * H])
            b0["oe_nat"][k] = nat
            trp = ps_tr.tile([P128, H], f16, tag="tr", name="tr_oe")
            for hc in range(NH):
                nc.tensor.transpose(
                    trp[:, hc * P128:(hc + 1) * P128],
                    nat[:, hc * P128:(hc + 1) * P128],
                    id16,
                )
            dst = b0["oeT_c"][k // 4][:, :, (k % 4) * P128:(k % 4 + 1) * P128]
            src = trp.rearrange("p (h s) -> p h s", h=NH)
            if k % 2 == 0:
                nc.scalar.copy(dst, src)
            else:
                nc.vector.tensor_copy(dst, src)

        def prep_od0(k):
            odr = loads.tile([P128, H], f32, tag="odr", name="odr")
            nc.sync.dma_start(out=odr, in_=d_od[k * P128:(k + 1) * P128, 0, :])
            odf = loads.tile([P128, H], f16, tag="odf", name="odf")
            nc.vector.tensor_copy(odf, odr)
            trp2 = ps_tr.tile([P128, H], f16, tag="tr", name="tr_od")
            for hc in range(NH):
                nc.tensor.transpose(
                    trp2[:, hc * P128:(hc + 1) * P128],
                    odf[:, hc * P128:(hc + 1) * P128],
                    id16,
                )
            dst2 = b0["odT_t"][k][:, :, :]
            src2 = trp2.rearrange("p (h t) -> p h t", h=NH)
            if k % 2 == 0:
                nc.vector.tensor_copy(dst2, src2)
            else:
                nc.scalar.copy(dst2, src2)

        # ---- batch 1: crossbar prep (SP queue, DRAM fp16 bounce) ----
        b1 = {
            "b": 1,
            "oe_nat": [None] * NS,
            "oe16": dscratch.tile([S, H], f16, tag="oe16", name="oe16"),
            "od16": dscratch.tile([T, H], f16, tag="od16", name="od16"),
            "oeT_c": [
                persist.tile([P128, NH, SC], f16, tag=f"oeT{g}", name=f"oeT{g}")
                for g in range(NSC)
            ],
            "odT_g": [
                persist.tile([P128, NH, SC], f16, tag=f"odTg{g}", name=f"odTg{g}")
                for g in range(NT // GRP)
            ],
        }
        b1["odw"] = lambda tt, hc: b1["odT_g"][tt // GRP][
            :, hc, (tt % GRP) * P128:(tt % GRP + 1) * P128
        ]

        def prep_oe_group1(g):
            for k in range(GRP * g, GRP * g + GRP):
                raw = loads.tile([P128, 2 * H], f32, tag="raw", name="raw")
                nc.sync.dma_start(out=raw, in_=d_oe[k * P128:(k + 1) * P128, 1, :])
                nat = persist.tile([P128, H], f16, tag=f"oenat{k}", name=f"oenat{k}")
                nc.vector.tensor_add(nat, raw[:, 0:H], raw[:, H:2 * H])
                b1["oe_nat"][k] = nat
                nc.sync.dma_start(out=b1["oe16"][k * P128:(k + 1) * P128, :], in_=nat)
            nc.sync.dma_start_transpose(
                b1["oeT_c"][g], b1["oe16"][g * SC:(g + 1) * SC, :]
            )

        def prep_od_group1(g):
            for k in range(GRP * g, GRP * g + GRP):
                odr = loads.tile([P128, H], f32, tag="odr", name="odr")
                nc.sync.dma_start(out=odr, in_=d_od[k * P128:(k + 1) * P128, 1, :])
                odf = loads.tile([P128, H], f16, tag="odf", name="odf")
                nc.vector.tensor_copy(odf, odr)
                nc.sync.dma_start(out=b1["od16"][k * P128:(k + 1) * P128, :], in_=odf)
            nc.sync.dma_start_transpose(
                b1["odT_g"][g], b1["od16"][g * SC:(g + 1) * SC, :]
            )

        # ---- stages ----
        def stage1_begin(bs, tt):
            mx = small.tile([P128, NSC], f32, tag="mx", name="mx")
            return {"bs": bs, "tt": tt, "mx": mx, "ps_s": []}

        def stage1_chunk(st1, ci):
            bs, tt, mx = st1["bs"], st1["tt"], st1["mx"]
            if ci % 2 == 0:
                st1["ps_s"].append(
                    ps_s_pool.tile(
                        [P128, 2 * SC], f32, tag=f"ps_h{ci // 2}", name=f"ps_h{ci // 2}"
                    )
                )
            pss = st1["ps_s"][ci // 2][:, (ci % 2) * SC:(ci % 2 + 1) * SC]
            for hc in range(NH):
                nc.tensor.matmul(
                    pss,
                    bs["odw"](tt, hc),
                    bs["oeT_c"][ci][:, hc, :],
                    start=(hc == 0),
                    stop=(hc == NH - 1),
                )
            nc.vector.reduce_max(mx[:, ci:ci + 1], pss, axis=mybir.AxisListType.X)

        def stage1_exp(st1):
            """m/neg_m + the two exp halves; l/linv deferred to stage1_l."""
            bs, tt, mx, ps_s = st1["bs"], st1["tt"], st1["mx"], st1["ps_s"]
            neg_m = small.tile([P128, 1], f32, tag="neg_m", name="neg_m")
            m = small.tile([P128, 1], f32, tag="m", name="m")
            nc.vector.reduce_max(m, mx, axis=mybir.AxisListType.X)
            nc.vector.tensor_scalar_mul(neg_m, m, -1.0)

            lacc = small.tile([P128, 2], f32, tag="lacc", name="lacc")
            pts = ptile.tile([P128, S], f16, tag="pts", name="pts")
            for h2 in range(2):
                nc.scalar.activation(
                    pts[:, h2 * 2 * SC:(h2 + 1) * 2 * SC], ps_s[h2],
                    mybir.ActivationFunctionType.Exp,
                    bias=neg_m, scale=1.0,
                    accum_out=lacc[:, h2:h2 + 1],
                )
            st1["lacc"] = lacc
            st1["pts"] = pts

        def stage1_l(st1):
            l = small.tile([P128, 1], f32, tag="l", name="l")
            nc.vector.reduce_sum(l, st1["lacc"], axis=mybir.AxisListType.X)
            linv = small.tile([P128, 1], f32, tag="linv", name="linv")
            nc.vector.reciprocal(linv, l)
            return st1["bs"], st1["tt"], st1["pts"], linv

        def stage2(state):
            bs, tt, pts, linv = state
            b = bs["b"]
            pT_c = []
            ptr = None
            for ci in range(NSC):
                if ci % 2 == 0:
                    ptr = ps_tr.tile([P128, 2, SC], f16, tag="tr", name="ptr")
                half = ci % 2
                for j in range(SC // P128):
                    nc.tensor.transpose(
                        ptr[:, half, j * P128:(j + 1) * P128],
                        pts[:, ci * SC + j * P128:ci * SC + (j + 1) * P128],
                        id16,
                    )
                pc = pcpool.tile([P128, SC], f16, tag=f"pT{ci}", name=f"pT{ci}")
                nc.vector.tensor_copy(pc, ptr[:, half, :])
                pT_c.append(pc)

            ps_c = ps_cp.tile([P128, H], f32, tag="ps_c", name="ps_c")
            for k in range(NS):
                nc.tensor.matmul(
                    ps_c,
                    pT_c[k // 4][:, (k % 4) * P128:(k % 4 + 1) * P128],
                    bs["oe_nat"][k],
                    start=(k == 0), stop=(k == NS - 1),
                )
            ot = outs.tile([P128, H], f32, tag="ot", name="ot")
            nc.scalar.activation(
                ot, ps_c, mybir.ActivationFunctionType.Identity,
                bias=0.0, scale=linv,
            )
            nc.sync.dma_start(out=d_out[tt * P128:(tt + 1) * P128, b, :], in_=ot)

        # ---- flattened two-batch pipeline ----
        # batch 0 startup: per-tile prep interleaved with the first t-tile's
        # scores chunks (chunk ci needs oe tiles 4ci..4ci+3 and odT_t[0]).
        st1_0 = stage1_begin(b0, 0)
        for g in range(NSC):
            for k in range(4 * g, 4 * g + 4):
                prep_oe0(k)
            if g == 0:
                prep_od0(0)
            stage1_chunk(st1_0, g)
        prep_od0(1)
        stage1_exp(st1_0)
        states = [stage1_l(st1_0)]

        # b0 od tiles prepped one tile ahead; b1 group prep interleaved into
        # batch 0's tail (all on the SP queue).
        prep_at = {
            8: ("oe1", 0), 9: ("oe1", 1), 10: ("oe1", 2), 11: ("oe1", 3),
            12: ("od1", 0), 13: ("od1", 1), 18: ("od1", 2), 21: ("od1", 3),
        }
        for i in range(1, 2 * NT):
            bs, tt = (b0, i) if i < NT else (b1, i - NT)
            if i < NT - 1:
                prep_od0(i + 1)
            if i in prep_at:
                kind, g = prep_at[i]
                (prep_oe_group1 if kind == "oe1" else prep_od_group1)(g)
            st1 = stage1_begin(bs, tt)
            for ci in range(NSC):
                stage1_chunk(st1, ci)
            stage1_exp(st1)
            if len(states) >= 2:
                stage2(states.pop(0))
            states.append(stage1_l(st1))
        while states:
            stage2(states.pop(0))

    nc.finalize()
    return nc


def _ensure_devices():
    """Make sure the 8 NeuronCores are visible to jax.devices().

    The calling harness may have pinned jax to cpu (JAX_PLATFORMS=cpu is a
    common pin for running the jax reference); the Bass SPMD launcher uses
    jax.devices(), so re-point jax at the neuron platform if needed.
    """
    import os
    import jax

    try:
        devs = jax.devices()
    except Exception:
        devs = []
    if sum(1 for d in devs if d.platform != "cpu") >= NCORES:
        return
    for plats in ("axon,cpu", None):
        try:
            if plats is None:
                os.environ.pop("JAX_PLATFORMS", None)
            else:
                os.environ["JAX_PLATFORMS"] = plats
            jax.config.update("jax_platforms", plats)
            from jax.extend.backend import clear_backends

            clear_backends()
            devs = jax.devices()
            if sum(1 for d in devs if d.platform != "cpu") >= NCORES:
                return
        except Exception:
            continue


def kernel(in_e=None, out_e=None, out_d=None, **kwargs):
    global _cached_nc
    from concourse.bass_utils import run_bass_kernel_spmd

    _ensure_devices()

    out_e = np.asarray(out_e, dtype=np.float32)
    out_d = np.asarray(out_d, dtype=np.float32)
    if _cached_nc is None:
        _cached_nc = _build()
    in_maps = []
    for c in range(NCORES):
        bsl = slice(c * BLOC, (c + 1) * BLOC)
        in_maps.append({
            "out_e": np.ascontiguousarray(out_e[:, bsl, :]),
            "out_d": np.ascontiguousarray(out_d[:, bsl, :]),
        })
    res = run_bass_kernel_spmd(_cached_nc, in_maps, list(range(NCORES)))
    return np.concatenate([res.results[c]["out"] for c in range(NCORES)], axis=1)


# revision 23
# speedup vs baseline: 1.0592x; 1.0592x over previous
"""Trainium2 Bass kernel for nn_Attention_43516608643501.

Cross-attention: Q = out_d [T,B,H]; K = V = sum of fwd/bwd halves of out_e
-> [S,B,H]; scores = Q @ K^T per batch (contraction over H, no scaling);
softmax over the source dim S; context = P @ V -> output [T,B,H].

Sharding: data-parallel over batch (dim 1): 2 batches per core x 8 cores,
no cross-core communication.

Layout: scores are computed in [t_partition, s_free] tiles so the softmax
max and sum are free-dim reductions (DVE reduce_max + the ACT activation's
accum_out register). The per-row max makes the kernel robust to any input
realization (exp args <= 0, P in [0,1], l in [1,S]) and lets P live in
fp16. P is then transposed back to [s,t] blocks on the PE (fp16 transpose,
1 cyc/row) for the P^T @ V accumulation.

Numerics: both matmuls run in fp16 (full PE rate; fp16's 4.9e-4 rounding
vs bf16's 4e-3 matters because the scores carry no 1/sqrt(H) scaling, so
near-ties in the softmax amplify score error by exp()).
"""

import numpy as np
from contextlib import ExitStack

S, T, B, H = 2048, 2048, 16, 512
NCORES = 8
BLOC = B // NCORES  # batches per core
P128 = 128
NS = S // P128  # 16 s-tiles
NT = T // P128  # 16 t-tiles
NH = H // P128  # 4 h-chunks of the contraction
SC = 512  # s-chunk width (scores tile columns)
NSC = S // SC  # 4 s-chunks per t-tile

_cached_nc = None


def _build():
    import concourse.bacc as bacc
    import concourse.tile as tile
    from concourse import mybir
    from concourse.masks import make_identity

    f32 = mybir.dt.float32
    f16 = mybir.dt.float16

    nc = bacc.Bacc(None, target_bir_lowering=False)
    d_oe = nc.dram_tensor("out_e", [S, BLOC, 2 * H], f32, kind="ExternalInput")
    d_od = nc.dram_tensor("out_d", [T, BLOC, H], f32, kind="ExternalInput")
    d_out = nc.dram_tensor("out", [T, BLOC, H], f32, kind="ExternalOutput")

    with ExitStack() as ctx:
        tc = ctx.enter_context(tile.TileContext(nc))
        singles = ctx.enter_context(tc.tile_pool(name="singles", bufs=1))
        loads = ctx.enter_context(tc.tile_pool(name="loads", bufs=10))
        persist = ctx.enter_context(tc.tile_pool(name="persist", bufs=2))
        ptile = ctx.enter_context(tc.tile_pool(name="ptile", bufs=2))
        outs = ctx.enter_context(tc.tile_pool(name="outs", bufs=3))
        small = ctx.enter_context(tc.tile_pool(name="small", bufs=3))
        # PSUM: 8 banks = ps_s0..3 (4) + ptr (2) + ps_c (2)
        ps_s_pool = ctx.enter_context(tc.tile_pool(name="ps_s_pool", bufs=1, space="PSUM"))
        ps_tr = ctx.enter_context(tc.tile_pool(name="ps_tr", bufs=2, space="PSUM"))
        ps_cp = ctx.enter_context(tc.tile_pool(name="ps_cp", bufs=2, space="PSUM"))

        id16 = singles.tile([P128, P128], f16)
        make_identity(nc, id16)

        for b in range(BLOC):
            # ---- prep: oe halves summed to fp16 (V and transpose source);
            # oeT/odT = h-on-partition layouts for the scores matmul.
            # Interleave oe/od tiles and keep the transposed tensors
            # chunk-granular so the first scores matmul only depends on the
            # first few loads, not on the whole prep phase. ----
            oe_nat = []
            oeT_c = [
                persist.tile([P128, NH, SC], f16, tag=f"oeT{ci}", name=f"oeT{ci}")
                for ci in range(NSC)
            ]
            odT_t = [
                persist.tile([P128, NH, P128], f16, tag=f"odT{tt}", name=f"odT{tt}")
                for tt in range(NT)
            ]
            def prep_oe(k):
                raw = loads.tile([P128, 2 * H], f32, tag="raw", name="raw")
                nc.sync.dma_start(
                    out=raw, in_=d_oe[k * P128:(k + 1) * P128, b, :]
                )
                nat = persist.tile(
                    [P128, H], f16, tag=f"oenat{k}", name=f"oenat{k}"
                )
                nc.vector.tensor_add(nat, raw[:, 0:H], raw[:, H:2 * H])
                oe_nat.append(nat)
                trp = ps_tr.tile([P128, H], f16, tag="tr", name="tr_oe")
                for hc in range(NH):
                    nc.tensor.transpose(
                        trp[:, hc * P128:(hc + 1) * P128],
                        nat[:, hc * P128:(hc + 1) * P128],
                        id16,
                    )
                dst = oeT_c[k // 4][:, :, (k % 4) * P128:(k % 4 + 1) * P128]
                src = trp.rearrange("p (h s) -> p h s", h=NH)
                if k % 2 == 0:
                    nc.scalar.copy(dst, src)
                else:
                    nc.vector.tensor_copy(dst, src)

            def prep_od(k):
                odr = loads.tile([P128, H], f32, tag="odr", name="odr")
                nc.sync.dma_start(
                    out=odr, in_=d_od[k * P128:(k + 1) * P128, b, :]
                )
                odf = loads.tile([P128, H], f16, tag="odf", name="odf")
                nc.vector.tensor_copy(odf, odr)
                trp2 = ps_tr.tile([P128, H], f16, tag="tr", name="tr_od")
                for hc in range(NH):
                    nc.tensor.transpose(
                        trp2[:, hc * P128:(hc + 1) * P128],
                        odf[:, hc * P128:(hc + 1) * P128],
                        id16,
                    )
                dst2 = odT_t[k][:, :, :]
                src2 = trp2.rearrange("p (h t) -> p h t", h=NH)
                if k % 2 == 0:
                    nc.vector.tensor_copy(dst2, src2)
                else:
                    nc.scalar.copy(dst2, src2)


            # ---- main: per t-tile of 128 query rows, software-pipelined:
            # stage 1 (tile tt): scores matmuls + max + exp(P);
            # stage 2 (tile tt-1): P transposes, PSUM->SBUF copies, P^T @ V.
            # PE alternates mm1(tt) / tr+mm2(tt-1) so the softmax
            # (DVE reductions + ACT exp) of tt hides under PE work. ----
            def stage1_begin(tt):
                mx = small.tile([P128, NSC], f32, tag="mx", name="mx")
                return {"tt": tt, "mx": mx, "ps_s": []}

            def stage1_chunk(st1, ci):
                tt, mx = st1["tt"], st1["mx"]
                pss = ps_s_pool.tile(
                    [P128, SC], f32, tag=f"ps_s{ci}", name=f"ps_s{ci}"
                )
                for hc in range(NH):
                    nc.tensor.matmul(
                        pss,
                        odT_t[tt][:, hc, :],
                        oeT_c[ci][:, hc, :],
                        start=(hc == 0),
                        stop=(hc == NH - 1),
                    )
                nc.vector.reduce_max(
                    mx[:, ci:ci + 1], pss, axis=mybir.AxisListType.X
                )
                st1["ps_s"].append(pss)

            def stage1_finish(st1):
                """m/neg_m + exp only; l/linv are deferred into stage2 so
                the DVE stream of the NEXT iteration runs [reduces, P-copies,
                l, linv] instead of parking the copies behind an l that
                waits on the whole exp chain."""
                tt, mx, ps_s = st1["tt"], st1["mx"], st1["ps_s"]
                neg_m = small.tile([P128, 1], f32, tag="neg_m", name="neg_m")
                m = small.tile([P128, 1], f32, tag="m", name="m")
                nc.vector.reduce_max(m, mx, axis=mybir.AxisListType.X)
                nc.vector.tensor_scalar_mul(neg_m, m, -1.0)

                lacc = small.tile([P128, NSC], f32, tag="lacc", name="lacc")
                pts = []
                for ci in range(NSC):
                    pt = ptile.tile([P128, SC], f16, tag=f"pt{ci}", name=f"pt{ci}")
                    nc.scalar.activation(
                        pt, ps_s[ci], mybir.ActivationFunctionType.Exp,
                        bias=neg_m, scale=1.0,
                        accum_out=lacc[:, ci:ci + 1],
                    )
                    pts.append(pt)
                st1["lacc"] = lacc
                st1["pts"] = pts
                return st1

            def stage1(tt):
                st1 = stage1_begin(tt)
                for ci in range(NSC):
                    stage1_chunk(st1, ci)
                return stage1_finish(st1)

            def stage2(st1_prev):
                tt, pts = st1_prev["tt"], st1_prev["pts"]
                tb = slice(tt * P128, (tt + 1) * P128)
                pT_c = []
                ptr = None
                for ci in range(NSC):
                    if ci % 2 == 0:
                        ptr = ps_tr.tile([P128, 2, SC], f16, tag="tr", name="ptr")
                    half = ci % 2
                    for j in range(SC // P128):
                        nc.tensor.transpose(
                            ptr[:, half, j * P128:(j + 1) * P128],
                            pts[ci][:, j * P128:(j + 1) * P128],
                            id16,
                        )
                    # all copies on DVE: on ACT they queue behind this
                    # iteration's exp, which stalls the P^T @ V weight loads
                    pc = ptile.tile([P128, SC], f16, tag=f"pT{ci}", name=f"pT{ci}")
                    nc.vector.tensor_copy(pc, ptr[:, half, :])
                    pT_c.append(pc)

                ps_c = ps_cp.tile([P128, H], f32, tag="ps_c", name="ps_c")
                for k in range(NS):
                    nc.tensor.matmul(
                        ps_c,
                        pT_c[k // 4][:, (k % 4) * P128:(k % 4 + 1) * P128],
                        oe_nat[k],
                        start=(k == 0), stop=(k == NS - 1),
                    )
                # deferred softmax denominator: exp(tt) finished an iteration
                # ago, so l/linv issue after the copies and complete at once
                l = small.tile([P128, 1], f32, tag="l", name="l")
                nc.vector.reduce_sum(l, st1_prev["lacc"], axis=mybir.AxisListType.X)
                linv = small.tile([P128, 1], f32, tag="linv", name="linv")
                nc.vector.reciprocal(linv, l)
                ot = outs.tile([P128, H], f32, tag="ot", name="ot")
                nc.scalar.activation(
                    ot, ps_c, mybir.ActivationFunctionType.Identity,
                    bias=0.0, scale=linv,
                )
                nc.sync.dma_start(out=d_out[tb, b, :], in_=ot)

            # interleave the first t-tile's scores chunks into the oe prep:
            # chunk ci only needs oe tiles 4ci..4ci+3, so the PE starts real
            # work while later oe tiles are still loading.
            st1_0 = stage1_begin(0)
            for g in range(NSC):
                for k in range(4 * g, 4 * g + 4):
                    prep_oe(k)
                if g == 0:
                    prep_od(0)
                stage1_chunk(st1_0, g)
            prep_od(1)

            prev = stage1_finish(st1_0)
            for tt in range(1, NT):
                if tt + 1 < NT:
                    prep_od(tt + 1)
                state = stage1(tt)
                stage2(prev)
                prev = state
            stage2(prev)

    nc.finalize()
    return nc


def _ensure_devices():
    """Make sure the 8 NeuronCores are visible to jax.devices().

    The calling harness may have pinned jax to cpu (JAX_PLATFORMS=cpu is a
    common pin for running the jax reference); the Bass SPMD launcher uses
    jax.devices(), so re-point jax at the neuron platform if needed.
    """
    import os
    import jax

    try:
        devs = jax.devices()
    except Exception:
        devs = []
    if sum(1 for d in devs if d.platform != "cpu") >= NCORES:
        return
    for plats in ("axon,cpu", None):
        try:
            if plats is None:
                os.environ.pop("JAX_PLATFORMS", None)
            else:
                os.environ["JAX_PLATFORMS"] = plats
            jax.config.update("jax_platforms", plats)
            from jax.extend.backend import clear_backends

            clear_backends()
            devs = jax.devices()
            if sum(1 for d in devs if d.platform != "cpu") >= NCORES:
                return
        except Exception:
            continue


def kernel(in_e=None, out_e=None, out_d=None, **kwargs):
    global _cached_nc
    from concourse.bass_utils import run_bass_kernel_spmd

    _ensure_devices()

    out_e = np.asarray(out_e, dtype=np.float32)
    out_d = np.asarray(out_d, dtype=np.float32)
    if _cached_nc is None:
        _cached_nc = _build()
    in_maps = []
    for c in range(NCORES):
        bsl = slice(c * BLOC, (c + 1) * BLOC)
        in_maps.append({
            "out_e": np.ascontiguousarray(out_e[:, bsl, :]),
            "out_d": np.ascontiguousarray(out_d[:, bsl, :]),
        })
    res = run_bass_kernel_spmd(_cached_nc, in_maps, list(range(NCORES)))
    return np.concatenate([res.results[c]["out"] for c in range(NCORES)], axis=1)



# revision 25
# speedup vs baseline: 1.0594x; 1.0001x over previous
"""Trainium2 Bass kernel for nn_Attention_43516608643501.

Cross-attention: Q = out_d [T,B,H]; K = V = sum of fwd/bwd halves of out_e
-> [S,B,H]; scores = Q @ K^T per batch (contraction over H, no scaling);
softmax over the source dim S; context = P @ V -> output [T,B,H].

Sharding: data-parallel over batch (dim 1): 2 batches per core x 8 cores,
no cross-core communication.

Layout: scores are computed in [t_partition, s_free] tiles so the softmax
max and sum are free-dim reductions (DVE reduce_max + the ACT activation's
accum_out register). The per-row max makes the kernel robust to any input
realization (exp args <= 0, P in [0,1], l in [1,S]) and lets P live in
fp16. P is then transposed back to [s,t] blocks on the PE (fp16 transpose,
1 cyc/row) for the P^T @ V accumulation.

Numerics: both matmuls run in fp16 (full PE rate; fp16's 4.9e-4 rounding
vs bf16's 4e-3 matters because the scores carry no 1/sqrt(H) scaling, so
near-ties in the softmax amplify score error by exp()).
"""

import numpy as np
from contextlib import ExitStack

S, T, B, H = 2048, 2048, 16, 512
NCORES = 8
BLOC = B // NCORES  # batches per core
P128 = 128
NS = S // P128  # 16 s-tiles
NT = T // P128  # 16 t-tiles
NH = H // P128  # 4 h-chunks of the contraction
SC = 512  # s-chunk width (scores tile columns)
NSC = S // SC  # 4 s-chunks per t-tile

_cached_nc = None


def _build():
    import concourse.bacc as bacc
    import concourse.tile as tile
    from concourse import mybir
    from concourse.masks import make_identity

    f32 = mybir.dt.float32
    f16 = mybir.dt.float16

    nc = bacc.Bacc(None, target_bir_lowering=False)
    d_oe = nc.dram_tensor("out_e", [S, BLOC, 2 * H], f32, kind="ExternalInput")
    d_od = nc.dram_tensor("out_d", [T, BLOC, H], f32, kind="ExternalInput")
    d_out = nc.dram_tensor("out", [T, BLOC, H], f32, kind="ExternalOutput")

    with ExitStack() as ctx:
        tc = ctx.enter_context(tile.TileContext(nc))
        singles = ctx.enter_context(tc.tile_pool(name="singles", bufs=1))
        loads = ctx.enter_context(tc.tile_pool(name="loads", bufs=10))
        persist = ctx.enter_context(tc.tile_pool(name="persist", bufs=2))
        ptile = ctx.enter_context(tc.tile_pool(name="ptile", bufs=2))
        outs = ctx.enter_context(tc.tile_pool(name="outs", bufs=3))
        small = ctx.enter_context(tc.tile_pool(name="small", bufs=3))
        # PSUM: 8 banks = ps_s0..3 (4) + ptr (2) + ps_c (2)
        ps_s_pool = ctx.enter_context(tc.tile_pool(name="ps_s_pool", bufs=1, space="PSUM"))
        ps_tr = ctx.enter_context(tc.tile_pool(name="ps_tr", bufs=2, space="PSUM"))
        ps_cp = ctx.enter_context(tc.tile_pool(name="ps_cp", bufs=2, space="PSUM"))

        id16 = singles.tile([P128, P128], f16)
        make_identity(nc, id16)

        for b in range(BLOC):
            # ---- prep: oe halves summed to fp16 (V and transpose source);
            # oeT/odT = h-on-partition layouts for the scores matmul.
            # Interleave oe/od tiles and keep the transposed tensors
            # chunk-granular so the first scores matmul only depends on the
            # first few loads, not on the whole prep phase. ----
            oe_nat = []
            oeT_c = [
                persist.tile([P128, NH, SC], f16, tag=f"oeT{ci}", name=f"oeT{ci}")
                for ci in range(NSC)
            ]
            odT_t = [
                persist.tile([P128, NH, P128], f16, tag=f"odT{tt}", name=f"odT{tt}")
                for tt in range(NT)
            ]
            def prep_oe(k):
                raw = loads.tile([P128, 2 * H], f32, tag="raw", name="raw")
                nc.sync.dma_start(
                    out=raw, in_=d_oe[k * P128:(k + 1) * P128, b, :]
                )
                nat = persist.tile(
                    [P128, H], f16, tag=f"oenat{k}", name=f"oenat{k}"
                )
                nc.vector.tensor_add(nat, raw[:, 0:H], raw[:, H:2 * H])
                oe_nat.append(nat)
                trp = ps_tr.tile([P128, H], f16, tag="tr", name="tr_oe")
                for hc in range(NH):
                    nc.tensor.transpose(
                        trp[:, hc * P128:(hc + 1) * P128],
                        nat[:, hc * P128:(hc + 1) * P128],
                        id16,
                    )
                dst = oeT_c[k // 4][:, :, (k % 4) * P128:(k % 4 + 1) * P128]
                src = trp.rearrange("p (h s) -> p h s", h=NH)
                if k % 2 == 0:
                    nc.scalar.copy(dst, src)
                else:
                    nc.vector.tensor_copy(dst, src)

            def prep_od(k):
                odr = loads.tile([P128, H], f32, tag="odr", name="odr")
                nc.sync.dma_start(
                    out=odr, in_=d_od[k * P128:(k + 1) * P128, b, :]
                )
                odf = loads.tile([P128, H], f16, tag="odf", name="odf")
                nc.vector.tensor_copy(odf, odr)
                trp2 = ps_tr.tile([P128, H], f16, tag="tr", name="tr_od")
                for hc in range(NH):
                    nc.tensor.transpose(
                        trp2[:, hc * P128:(hc + 1) * P128],
                        odf[:, hc * P128:(hc + 1) * P128],
                        id16,
                    )
                dst2 = odT_t[k][:, :, :]
                src2 = trp2.rearrange("p (h t) -> p h t", h=NH)
                if k % 2 == 0:
                    nc.vector.tensor_copy(dst2, src2)
                else:
                    nc.scalar.copy(dst2, src2)


            # ---- main: per t-tile of 128 query rows, software-pipelined:
            # stage 1 (tile tt): scores matmuls + max + exp(P);
            # stage 2 (tile tt-1): P transposes, PSUM->SBUF copies, P^T @ V.
            # PE alternates mm1(tt) / tr+mm2(tt-1) so the softmax
            # (DVE reductions + ACT exp) of tt hides under PE work. ----
            def stage1_begin(tt):
                mx = small.tile([P128, NSC], f32, tag="mx", name="mx")
                return {"tt": tt, "mx": mx, "ps_s": []}

            def stage1_chunk(st1, ci):
                tt, mx = st1["tt"], st1["mx"]
                pss = ps_s_pool.tile(
                    [P128, SC], f32, tag=f"ps_s{ci}", name=f"ps_s{ci}"
                )
                for hc in range(NH):
                    nc.tensor.matmul(
                        pss,
                        odT_t[tt][:, hc, :],
                        oeT_c[ci][:, hc, :],
                        start=(hc == 0),
                        stop=(hc == NH - 1),
                    )
                nc.vector.reduce_max(
                    mx[:, ci:ci + 1], pss, axis=mybir.AxisListType.X
                )
                st1["ps_s"].append(pss)

            def stage1_finish(st1):
                tt, mx, ps_s = st1["tt"], st1["mx"], st1["ps_s"]
                neg_m = small.tile([P128, 1], f32, tag="neg_m", name="neg_m")
                m = small.tile([P128, 1], f32, tag="m", name="m")
                nc.vector.reduce_max(m, mx, axis=mybir.AxisListType.X)
                nc.vector.tensor_scalar_mul(neg_m, m, -1.0)

                lacc = small.tile([P128, NSC], f32, tag="lacc", name="lacc")
                pts = []
                for ci in range(NSC):
                    pt = ptile.tile([P128, SC], f16, tag=f"pt{ci}", name=f"pt{ci}")
                    nc.scalar.activation(
                        pt, ps_s[ci], mybir.ActivationFunctionType.Exp,
                        bias=neg_m, scale=1.0,
                        accum_out=lacc[:, ci:ci + 1],
                    )
                    pts.append(pt)
                l = small.tile([P128, 1], f32, tag="l", name="l")
                nc.vector.reduce_sum(l, lacc, axis=mybir.AxisListType.X)
                linv = small.tile([P128, 1], f32, tag="linv", name="linv")
                nc.vector.reciprocal(linv, l)
                return tt, pts, linv

            def stage1(tt):
                # hc-outer loop order: the stationary tile odT_t[tt][:,hc,:]
                # is shared by 4 consecutive matmuls (one per s-chunk PSUM
                # bank), so redundant LDWEIGHTS can be elided. Numerically
                # identical to ci-outer; the softmax still waits on all four
                # chunk maxes either way.
                st1 = stage1_begin(tt)
                mx = st1["mx"]
                for ci in range(NSC):
                    st1["ps_s"].append(
                        ps_s_pool.tile(
                            [P128, SC], f32, tag=f"ps_s{ci}", name=f"ps_s{ci}"
                        )
                    )
                for hc in range(NH):
                    for ci in range(NSC):
                        nc.tensor.matmul(
                            st1["ps_s"][ci],
                            odT_t[tt][:, hc, :],
                            oeT_c[ci][:, hc, :],
                            start=(hc == 0),
                            stop=(hc == NH - 1),
                        )
                for ci in range(NSC):
                    nc.vector.reduce_max(
                        mx[:, ci:ci + 1], st1["ps_s"][ci],
                        axis=mybir.AxisListType.X,
                    )
                return stage1_finish(st1)

            def stage2(state):
                tt, pts, linv = state
                tb = slice(tt * P128, (tt + 1) * P128)
                pT_c = []
                ptr = None
                for ci in range(NSC):
                    if ci % 2 == 0:
                        ptr = ps_tr.tile([P128, 2, SC], f16, tag="tr", name="ptr")
                    half = ci % 2
                    for j in range(SC // P128):
                        nc.tensor.transpose(
                            ptr[:, half, j * P128:(j + 1) * P128],
                            pts[ci][:, j * P128:(j + 1) * P128],
                            id16,
                        )
                    pc = ptile.tile([P128, SC], f16, tag=f"pT{ci}", name=f"pT{ci}")
                    if ci < 2:
                        nc.scalar.copy(pc, ptr[:, half, :])
                    else:
                        nc.vector.tensor_copy(pc, ptr[:, half, :])
                    pT_c.append(pc)

                ps_c = ps_cp.tile([P128, H], f32, tag="ps_c", name="ps_c")
                for k in range(NS):
                    nc.tensor.matmul(
                        ps_c,
                        pT_c[k // 4][:, (k % 4) * P128:(k % 4 + 1) * P128],
                        oe_nat[k],
                        start=(k == 0), stop=(k == NS - 1),
                    )
                ot = outs.tile([P128, H], f32, tag="ot", name="ot")
                nc.scalar.activation(
                    ot, ps_c, mybir.ActivationFunctionType.Identity,
                    bias=0.0, scale=linv,
                )
                nc.sync.dma_start(out=d_out[tb, b, :], in_=ot)

            # interleave the first t-tile's scores chunks into the oe prep:
            # chunk ci only needs oe tiles 4ci..4ci+3, so the PE starts real
            # work while later oe tiles are still loading.
            st1_0 = stage1_begin(0)
            for g in range(NSC):
                for k in range(4 * g, 4 * g + 4):
                    prep_oe(k)
                if g == 0:
                    prep_od(0)
                stage1_chunk(st1_0, g)
            prep_od(1)

            prev = stage1_finish(st1_0)
            for tt in range(1, NT):
                if tt + 1 < NT:
                    prep_od(tt + 1)
                state = stage1(tt)
                stage2(prev)
                prev = state
            stage2(prev)

    nc.finalize()
    return nc


def _ensure_devices():
    """Make sure the 8 NeuronCores are visible to jax.devices().

    The calling harness may have pinned jax to cpu (JAX_PLATFORMS=cpu is a
    common pin for running the jax reference); the Bass SPMD launcher uses
    jax.devices(), so re-point jax at the neuron platform if needed.
    """
    import os
    import jax

    try:
        devs = jax.devices()
    except Exception:
        devs = []
    if sum(1 for d in devs if d.platform != "cpu") >= NCORES:
        return
    for plats in ("axon,cpu", None):
        try:
            if plats is None:
                os.environ.pop("JAX_PLATFORMS", None)
            else:
                os.environ["JAX_PLATFORMS"] = plats
            jax.config.update("jax_platforms", plats)
            from jax.extend.backend import clear_backends

            clear_backends()
            devs = jax.devices()
            if sum(1 for d in devs if d.platform != "cpu") >= NCORES:
                return
        except Exception:
            continue


def kernel(in_e=None, out_e=None, out_d=None, **kwargs):
    global _cached_nc
    from concourse.bass_utils import run_bass_kernel_spmd

    _ensure_devices()

    out_e = np.asarray(out_e, dtype=np.float32)
    out_d = np.asarray(out_d, dtype=np.float32)
    if _cached_nc is None:
        _cached_nc = _build()
    in_maps = []
    for c in range(NCORES):
        bsl = slice(c * BLOC, (c + 1) * BLOC)
        in_maps.append({
            "out_e": np.ascontiguousarray(out_e[:, bsl, :]),
            "out_d": np.ascontiguousarray(out_d[:, bsl, :]),
        })
    res = run_bass_kernel_spmd(_cached_nc, in_maps, list(range(NCORES)))
    return np.concatenate([res.results[c]["out"] for c in range(NCORES)], axis=1)

